# revision 9
# baseline (speedup 1.0000x reference)
"""Distributed causal attention block (QKV + RoPE + SDPA + Wo) on 8 TRN2 cores.

Sharding: tensor-parallel over heads (2 heads/core). Each core:
  phase 1: weight-stationary transposed QKV: q^T/k^T/v^T = Wqkv_c @ x^T
           streamed from host-pretransposed x^T (bf16); RoPE applied in the
           [e, t] layout with per-head even/odd partition split; v^T is
           PE-transposed back to [t, e] for the PV matmuls
  phase 2: causal attention per (batch, head) with TRANSPOSED scores
           s^T[k, q] (no P transposes); softmax sums via all-ones matmul
           (broadcast across partitions); 1/sum folded into the o^T drain
  phase 3: AllGather attention outputs (progressive pieces) -> Wo e-slice,
           emitted interleaved with phase 2 to avoid a serial tail
Host concatenates the 8 e-slices.

The q/k rows of Wqkv (and cos/sin tables) are permuted head-major
even/odd on the host; attention scores are invariant to a shared
permutation of the head dim of Q and K.
"""
import numpy as np
import ml_dtypes
import bass_rust
import concourse.bass as bass
import concourse.mybir as mybir
from concourse.tile import TileContext, add_dep_helper
from concourse.masks import make_identity

B, L, D, H = 2, 2048, 2048, 16
HD = 128
N_CORES = 8
HPC = H // N_CORES          # heads per core = 2
ES = HPC * HD               # 256 = e-slice width per core
T = B * L                   # 4096 tokens total
P = 128
SCALE = 1.0 / float(np.sqrt(HD))
NEG = -30000.0              # causal mask fill; exp(SCALE*(s+NEG)) underflows to 0
FP = mybir.dt.float32
BF = mybir.dt.bfloat16

N_TT = T // P               # 32 global t-tiles
N_LT = L // P               # 16 t-tiles per batch
N_DT = D // P               # 16 d-tiles
NQC = 4                     # 512-token q-chunks per batch

# attention-out AllGather pieces per batch, in units of 512-t q-chunks
AG_PIECES = {0: [(0, 2), (2, 4)], 1: [(0, 2), (2, 3), (3, 4)]}

# phase-2 block geometry: for (kt, qc) with qc >= kt//4:
#   off  = max(0, kt*128 - qc*512)   (column offset within the qc chunk)
#   w    = 512 - off
# blocks laid out kt-major in the expST tile
BLOCKS = []          # (kt, qc, off, w, boff)
_boff = 0
for _kt in range(16):
    for _qc in range(_kt // 4, 4):
        _off = max(0, _kt * 128 - _qc * 512)
        _w = 512 - _off
        BLOCKS.append((_kt, _qc, _off, _w, _boff))
        _boff += _w
EXP_COLS = _boff     # 17408
BLK = {(kt, qc): (off, w, boff) for (kt, qc, off, w, boff) in BLOCKS}


def split_multi_waits(nc):
    """This walrus build allows 1 sync wait per instruction (2 for
    EventSemaphore). Tile attaches more on some instructions (tail drain,
    collective-adjacent DMAs); hoist the extras onto same-engine NoOps."""
    for f in nc.m.functions:
        for bb in f.blocks:
            new_insts = []
            changed = False
            for ins in bb.instructions:
                si = ins.sync_info
                cap = 2 if type(ins).__name__ == "InstEventSemaphore" else 1
                if si is not None and len(si.on_wait) > cap:
                    waits = list(si.on_wait)
                    for k, w in enumerate(waits[cap:]):
                        new_insts.append(mybir.InstNoOp(
                            name=f"{ins.name}-wsplit{k}", ins=[], outs=[],
                            engine=ins.engine,
                            sync_info=bass_rust.SyncInfo(on_wait=[w], on_update=[]),
                        ))
                    ins.sync_info = bass_rust.SyncInfo(
                        on_wait=waits[:cap], on_update=list(si.on_update))
                    changed = True
                new_insts.append(ins)
            if changed:
                bb.instructions.clear()
                for i2 in new_insts:
                    bb.add_instruction(i2)


def make_causal_mask_T(nc, ap, mask_val):
    """mask[k, q] = 0 if k <= q else mask_val (transposed causal)."""
    sq = ap.shape[0]
    nc.gpsimd.memset(ap, 0.0)
    nc.gpsimd.affine_select(
        out=ap, in_=ap,
        compare_op=mybir.AluOpType.is_ge,
        fill=mask_val, base=0,
        # keep where (-x + y) >= 0, i.e. k <= q
        pattern=[[1, sq]],
        channel_multiplier=-1,
    )


def build(debug=False, fix_waits=True):
    nc = bass.Bass()
    xT = nc.declare_dram_parameter("xT", [D, T], BF, isOutput=False)
    wqkvT = nc.declare_dram_parameter("wqkvT", [D, 3 * ES], BF, isOutput=False)
    # per-head stacked trig tables: rows 0:64 = even-col table, 64:128 = odd
    cc_p = [nc.declare_dram_parameter(f"cc{h}", [P, L], FP, isOutput=False)
            for h in range(HPC)]
    ss_p = [nc.declare_dram_parameter(f"ss{h}", [P, L], FP, isOutput=False)
            for h in range(HPC)]
    woT = nc.declare_dram_parameter("woT", [D, ES], BF, isOutput=False)
    out = nc.declare_dram_parameter("out", [ES, T], FP, isOutput=True)
    if debug:
        dbg_qt = nc.declare_dram_parameter("dbg_qt", [P, HPC * T], FP, isOutput=True)
        dbg_kt = nc.declare_dram_parameter("dbg_kt", [P, HPC * T], FP, isOutput=True)
        dbg_v = nc.declare_dram_parameter("dbg_v", [P, N_TT * ES], FP, isOutput=True)
        dbg_ob = nc.declare_dram_parameter("dbg_ob", [P, B * HPC * L], FP,
                                           isOutput=True)

    o_bounce, ag_o = {}, {}
    for b, pieces in AG_PIECES.items():
        for (c0, c1) in pieces:
            w = (c1 - c0) * 512
            o_bounce[(b, c0)] = nc.dram_tensor(f"o_bounce{b}_{c0}", [ES, w], BF)
            ag_o[(b, c0)] = nc.dram_tensor(f"ag_o{b}_{c0}", [N_CORES * ES, w], BF,
                                           addr_space="Shared")
    rg = [list(range(N_CORES))]

    with TileContext(nc, pool_alloc_mode="queue") as tc:
        with (
            tc.tile_pool(name="const", bufs=1) as const_pool,
            tc.tile_pool(name="resident", bufs=1) as res_pool,
            tc.tile_pool(name="wo", bufs=1) as wo_pool,
            tc.tile_pool(name="vt", bufs=1) as vt_pool,
        ):
            ident = const_pool.tile([P, P], BF, name="ident")
            make_identity(nc, ident[:, :])
            cmT = const_pool.tile([P, P], FP, name="cmT")
            make_causal_mask_T(nc, cmT[:, :], NEG)
            ones = const_pool.tile([P, P], BF, name="ones")
            nc.gpsimd.memset(ones[:, :], 1.0)

            # resident through phases 1-2
            qt_sb = res_pool.tile([P, HPC * T], BF, name="qt_sb")   # [hd', h*T+t]
            kt_sb = res_pool.tile([P, HPC * T], BF, name="kt_sb")
            v_sb = res_pool.tile([P, N_TT * ES], BF, name="v_sb")   # [t%128, tt*ES+e]
            woT_sb = wo_pool.tile([P, N_DT * ES], BF, name="woT_sb")

            # ---------------- phase 1: transposed QKV + RoPE ----------------
            # eb order: q-h0, q-h1, k-h0, k-h1, v-0, v-1
            with (
                tc.tile_pool(name="wq", bufs=1) as wq_pool,
                tc.tile_pool(name="xt", bufs=1) as xt_pool,
                tc.tile_pool(name="rsc", bufs=1) as rsc_pool,
                tc.tile_pool(name="psG", bufs=2, space="PSUM") as psG,
            ):
                wt_sb = wq_pool.tile([P, N_DT * 3 * ES], BF, name="wt_sb")
                cc_sb = [wq_pool.tile([P, L], FP, name=f"cc{h}_sb")
                         for h in range(HPC)]
                ss_sb = [wq_pool.tile([P, L], FP, name=f"ss{h}_sb")
                         for h in range(HPC)]
                xt_sb = xt_pool.tile([P, N_DT * 2048], BF, name="xt_sb")
                vt_sb = vt_pool.tile([P, HPC * T], BF,
                                     name="vt_sb")  # [e, eb*T + t]

                # DMA priority: x^T th0 tiles + weights interleaved, then trig
                for dt in range(N_DT):
                    nc.sync.dma_start(
                        out=xt_sb[:, dt * 2048:(dt + 1) * 2048],
                        in_=xT[dt * P:(dt + 1) * P, 0:2048])
                    nc.sync.dma_start(
                        out=wt_sb[:, dt * 3 * ES:(dt + 1) * 3 * ES],
                        in_=wqkvT[dt * P:(dt + 1) * P, :])
                for h in range(HPC):
                    nc.sync.dma_start(out=cc_sb[h][:, :], in_=cc_p[h][:, :])
                    nc.sync.dma_start(out=ss_sb[h][:, :], in_=ss_p[h][:, :])

                def rope_drain(gp, dst, h, th):
                    cc, ss = cc_sb[h], ss_sb[h]
                    for c in range(4):
                        col = slice(c * 512, (c + 1) * 512)
                        e_ps, o_ps = gp[0:64, col], gp[64:128, col]
                        dcol = slice(h * T + th * 2048 + c * 512,
                                     h * T + th * 2048 + (c + 1) * 512)
                        t1 = rsc_pool.tile([64, 512], FP, name="t1", tag="t1")
                        t2 = rsc_pool.tile([64, 512], FP, name="t2", tag="t2")
                        nc.vector.tensor_tensor(t1[:, :], e_ps, cc[0:64, col],
                                                op=mybir.AluOpType.mult)
                        nc.vector.tensor_tensor(t2[:, :], o_ps, ss[0:64, col],
                                                op=mybir.AluOpType.mult)
                        nc.vector.tensor_tensor(dst[0:64, dcol], t1[:, :], t2[:, :],
                                                op=mybir.AluOpType.subtract)
                        t3 = rsc_pool.tile([64, 512], FP, name="t3", tag="t3")
                        t4 = rsc_pool.tile([64, 512], FP, name="t4", tag="t4")
                        nc.vector.tensor_tensor(t3[:, :], o_ps, cc[64:128, col],
                                                op=mybir.AluOpType.mult)
                        nc.vector.tensor_tensor(t4[:, :], e_ps, ss[64:128, col],
                                                op=mybir.AluOpType.mult)
                        nc.vector.tensor_tensor(dst[64:128, dcol], t3[:, :], t4[:, :],
                                                op=mybir.AluOpType.add)

                for th in range(2):
                    for ebi in range(6):
                        gp = psG.tile([P, 2048], FP, name="gp", tag="gp")
                        for dt in range(N_DT):
                            lhsT = wt_sb[:, dt * 3 * ES + ebi * P:
                                         dt * 3 * ES + (ebi + 1) * P]
                            for c in range(4):
                                nc.tensor.matmul(
                                    gp[:, c * 512:(c + 1) * 512], lhsT,
                                    xt_sb[:, dt * 2048 + c * 512:
                                          dt * 2048 + (c + 1) * 512],
                                    start=(dt == 0), stop=(dt == N_DT - 1))
                            if th == 0 and ebi == 5:
                                nc.sync.dma_start(
                                    out=xt_sb[:, dt * 2048:(dt + 1) * 2048],
                                    in_=xT[dt * P:(dt + 1) * P, 2048:4096])
                        if ebi < 2:
                            rope_drain(gp, qt_sb, ebi, th)
                        elif ebi < 4:
                            rope_drain(gp, kt_sb, ebi - 2, th)
                        else:
                            eb2 = ebi - 4
                            for c in range(4):
                                nc.scalar.copy(
                                    vt_sb[:, eb2 * T + th * 2048 + c * 512:
                                          eb2 * T + th * 2048 + (c + 1) * 512],
                                    gp[:, c * 512:(c + 1) * 512])

            # v^T -> v transposes (own PSUM scope, after psG frees)
            with tc.tile_pool(name="psT", bufs=2, space="PSUM") as psT:
                for eb2 in range(2):
                    for tg in range(8):  # groups of 4 of the 32 t-tiles
                        tr = psT.tile([P, 512], BF, name="tr", tag="tr")
                        for j in range(4):
                            tt_g = tg * 4 + j
                            nc.tensor.transpose(
                                tr[:, j * P:(j + 1) * P],
                                vt_sb[:, eb2 * T + tt_g * P:
                                      eb2 * T + (tt_g + 1) * P],
                                ident[:, :])
                        for j in range(4):
                            tt_g = tg * 4 + j
                            nc.vector.tensor_copy(
                                v_sb[:, tt_g * ES + eb2 * P:
                                     tt_g * ES + (eb2 + 1) * P],
                                tr[:, j * P:(j + 1) * P])

            if debug:
                with tc.tile_pool(name="dbgp", bufs=2) as dbgp:
                    for nm, src, dd in (("q", qt_sb, dbg_qt), ("k", kt_sb, dbg_kt),
                                        ("v", v_sb, dbg_v)):
                        for i in range(HPC * T // 512):
                            s = dbgp.tile([P, 512], FP, name="dstage")
                            nc.vector.tensor_copy(s[:, :],
                                                  src[:, i * 512:(i + 1) * 512])
                            nc.sync.dma_start(out=dd[:, i * 512:(i + 1) * 512],
                                              in_=s[:, :])

            # ---------------- phases 2+3 (interleaved) ----------------
            for dt in range(N_DT):
                nc.sync.dma_start(out=woT_sb[:, dt * ES:(dt + 1) * ES],
                                  in_=woT[dt * P:(dt + 1) * P, :])

            with (
                tc.tile_pool(name="pexp", bufs=2) as pexp,
                tc.tile_pool(name="prec", bufs=2) as prec,
                tc.tile_pool(name="p2ob", bufs=2) as p2ob,
                tc.tile_pool(name="p3x", bufs=2) as p3x,
                tc.tile_pool(name="p3o", bufs=4) as p3o,
                tc.tile_pool(name="psS", bufs=4, space="PSUM") as psS,
                tc.tile_pool(name="psSum", bufs=1, space="PSUM") as psSum,
                tc.tile_pool(name="psO", bufs=1, space="PSUM") as psO,
                tc.tile_pool(name="psW", bufs=2, space="PSUM") as psW,
            ):
                ob_tiles = {}
                pv_done = {}

                def scores_group(b, h, S, ktg):
                    """score blocks for k-tiles ktg*4..ktg*4+3 (kt-major)."""
                    qoff = h * T + b * L
                    for kt in range(ktg * 4, ktg * 4 + 4):
                        lhsT = kt_sb[:, qoff + kt * P: qoff + (kt + 1) * P]
                        for qc in range(kt // 4, 4):
                            off, w, boff = BLK[(kt, qc)]
                            sp = psS.tile([P, 512], FP, name="sp", tag="sp")
                            nc.tensor.matmul(
                                sp[:, :w], lhsT,
                                qt_sb[:, qoff + qc * 512 + off:
                                      qoff + (qc + 1) * 512],
                                start=True, stop=True)
                            if qc == kt // 4:  # leading 128 cols are diagonal
                                nc.vector.tensor_tensor(
                                    sp[:, 0:P], sp[:, 0:P], cmT[:, :],
                                    op=mybir.AluOpType.add)
                            nc.scalar.activation(
                                S[:, boff:boff + w], sp[:, :w],
                                mybir.ActivationFunctionType.Exp, scale=SCALE)

                def pv_chunk(b, h, qc, S, ob_sb):
                    """softmax-normalize + PV for one 512-q chunk."""
                    nkt = 4 * qc + 4
                    sm = psSum.tile([P, 512], FP, name="sm", tag="sm")
                    for kt in range(nkt):
                        off, w, boff = BLK[(kt, qc)]
                        nc.tensor.matmul(sm[:, off:], ones[:, :],
                                         S[:, boff:boff + w],
                                         start=(kt == 0), stop=(kt == nkt - 1))
                    rec = prec.tile([P, 512], FP, name="rec", tag="rec")
                    nc.vector.reciprocal(rec[:, :], sm[:, :])
                    o_ps = psO.tile([P, 512], FP, name="o_ps", tag="o")
                    for kt in range(nkt):
                        off, w, boff = BLK[(kt, qc)]
                        nc.tensor.matmul(
                            o_ps[:, off:],
                            v_sb[:, (b * N_LT + kt) * ES + h * HD:
                                 (b * N_LT + kt) * ES + (h + 1) * HD],
                            S[:, boff:boff + w],
                            start=(kt == 0), stop=(kt == nkt - 1))
                    obcp = nc.vector.tensor_tensor(
                        ob_sb[:, h * L + qc * 512:h * L + (qc + 1) * 512],
                        o_ps[:, :], rec[:, :], op=mybir.AluOpType.mult)
                    pv_done[(b, h, qc)] = obcp

                def ag_fire(b, c0, c1, ob_sb):
                    for h in range(HPC):
                        nc.sync.dma_start(
                            out=o_bounce[(b, c0)][h * HD:(h + 1) * HD, :],
                            in_=ob_sb[:, h * L + c0 * 512:h * L + c1 * 512])
                    nc.gpsimd.collective_compute(
                        "AllGather", mybir.AluOpType.bypass,
                        ins=[o_bounce[(b, c0)][:]],
                        outs=[ag_o[(b, c0)][:]],
                        replica_groups=rg)

                def phase2_qcwise(b, h, fire_pieces):
                    S = pexp.tile([P, EXP_COLS], BF, name="S", tag="S")
                    if h == 0:
                        ob_tiles[b] = p2ob.tile([P, HPC * L], BF,
                                                name="ob_sb", tag="ob")
                    ob_sb = ob_tiles[b]
                    for qc in range(4):
                        scores_group(b, h, S, qc)
                        pv_chunk(b, h, qc, S, ob_sb)
                        if fire_pieces:
                            for (c0, c1) in AG_PIECES[b]:
                                if c1 == qc + 1:
                                    ag_fire(b, c0, c1, ob_sb)

                def phase3(b, c0, c1):
                    w = (c1 - c0) * 512
                    nch = w // 512
                    ots = []
                    for tch in range(nch):
                        ot = p3x.tile([P, N_DT * 512], BF, name="ot", tag="ot")
                        for dt in range(N_DT):
                            nc.sync.dma_start(
                                out=ot[:, dt * 512:(dt + 1) * 512],
                                in_=ag_o[(b, c0)][dt * P:(dt + 1) * P,
                                                  tch * 512:(tch + 1) * 512])
                        ots.append(ot)
                    # same-weight pairing across the piece's chunks
                    for et in range(HPC):
                        fps = [psW.tile([P, 512], FP, name="f_ps", tag="f")
                               for _ in range(nch)]
                        for dt in range(N_DT):
                            lhsT = woT_sb[:, dt * ES + et * P:
                                          dt * ES + (et + 1) * P]
                            for i in range(nch):
                                nc.tensor.matmul(
                                    fps[i][:, :], lhsT,
                                    ots[i][:, dt * 512:(dt + 1) * 512],
                                    start=(dt == 0), stop=(dt == N_DT - 1))
                        for i in range(nch):
                            t0 = b * L + (c0 + i) * 512
                            f_sb = p3o.tile([P, 512], FP, name="f_sb")
                            nc.vector.tensor_copy(f_sb[:, :], fps[i][:, :])
                            nc.sync.dma_start(
                                out=out[et * P:(et + 1) * P, t0:t0 + 512],
                                in_=f_sb[:, :])

                phase2_qcwise(0, 0, fire_pieces=False)
                phase2_qcwise(0, 1, fire_pieces=True)
                phase2_qcwise(1, 0, fire_pieces=False)
                phase3(0, 0, 2)
                phase2_qcwise(1, 1, fire_pieces=True)
                phase3(0, 2, 4)
                phase3(1, 0, 2)
                phase3(1, 2, 3)
                phase3(1, 3, 4)

                if debug:
                    with tc.tile_pool(name="dbgo", bufs=2) as dbgo:
                        for b in range(B):
                            for i in range(HPC * L // 512):
                                s = dbgo.tile([P, 512], FP, name="dob")
                                nc.vector.tensor_copy(
                                    s[:, :],
                                    ob_tiles[b][:, i * 512:(i + 1) * 512])
                                nc.sync.dma_start(
                                    out=dbg_ob[:, b * HPC * L + i * 512:
                                               b * HPC * L + (i + 1) * 512],
                                    in_=s[:, :])

    if fix_waits:
        split_multi_waits(nc)
    return nc


def make_in_maps(x, cos, sin, Wqkv, Wo):
    bf = ml_dtypes.bfloat16
    xT_full = np.ascontiguousarray(
        np.asarray(x).reshape(T, D).T).astype(bf)
    # q/k row permutation: head-major, evens then odds
    perm = []
    for h in range(HPC):
        perm.extend(h * HD + 2 * np.arange(64))
        perm.extend(h * HD + 2 * np.arange(64) + 1)
    perm = np.asarray(perm)
    in_maps = []
    cosA, sinA = np.asarray(cos), np.asarray(sin)
    for c in range(N_CORES):
        cols = slice(c * ES, (c + 1) * ES)
        wq = Wqkv[c * ES:(c + 1) * ES, :][perm]
        wk = Wqkv[D + c * ES: D + (c + 1) * ES, :][perm]
        wv = Wqkv[2 * D + c * ES: 2 * D + (c + 1) * ES, :]
        w_c = np.concatenate([wq, wk, wv], axis=0)
        m = {
            "xT": xT_full,
            "wqkvT": np.ascontiguousarray(w_c.T.astype(bf)),
            "woT": np.ascontiguousarray(Wo[cols, :].T.astype(bf)),
        }
        for h in range(HPC):
            base = c * ES + h * HD
            ce = cosA[:, base + 2 * np.arange(64)].T      # [64, L]
            co = cosA[:, base + 2 * np.arange(64) + 1].T
            se = sinA[:, base + 2 * np.arange(64)].T
            so = sinA[:, base + 2 * np.arange(64) + 1].T
            m[f"cc{h}"] = np.ascontiguousarray(
                np.concatenate([ce, co], axis=0)).astype(np.float32)
            m[f"ss{h}"] = np.ascontiguousarray(
                np.concatenate([se, so], axis=0)).astype(np.float32)
        in_maps.append(m)
    return in_maps


_cache = {}


def kernel(x, cos, sin, Wqkv, Wo):
    from concourse.bass_utils import run_bass_kernel_spmd
    x = np.asarray(x, dtype=np.float32)
    cos = np.asarray(cos, dtype=np.float32)
    sin = np.asarray(sin, dtype=np.float32)
    Wqkv = np.asarray(Wqkv, dtype=np.float32)
    Wo = np.asarray(Wo, dtype=np.float32)
    if "nc" not in _cache:
        _cache["nc"] = build()
    nc = _cache["nc"]
    in_maps = make_in_maps(x, cos, sin, Wqkv, Wo)
    res = run_bass_kernel_spmd(nc, in_maps, core_ids=list(range(N_CORES)))
    pieces = [res.results[c]["out"].T for c in range(N_CORES)]
    return np.concatenate(pieces, axis=1).reshape(B, L, D)


# revision 11
# speedup vs baseline: 1.0882x; 1.0882x over previous
"""Distributed causal attention block (QKV + RoPE + SDPA + Wo) on 8 TRN2 cores.

Sharding: tensor-parallel over heads (2 heads/core). Each core:
  phase 1: weight-stationary transposed QKV: q^T/k^T/v^T = Wqkv_c @ x^T
           streamed from host-pretransposed x^T (bf16); RoPE applied in the
           [e, t] layout with per-head even/odd partition split; v^T is
           PE-transposed back to [t, e] for the PV matmuls
  phase 2: causal attention per (batch, head) with TRANSPOSED scores
           s^T[k, q] (no P transposes); softmax sums via all-ones matmul
           (broadcast across partitions); 1/sum folded into the o^T drain
  phase 3: AllGather attention outputs (progressive pieces) -> Wo e-slice,
           emitted interleaved with phase 2 to avoid a serial tail
Host concatenates the 8 e-slices.

The q/k rows of Wqkv (and cos/sin tables) are permuted head-major
even/odd on the host; attention scores are invariant to a shared
permutation of the head dim of Q and K.
"""
import numpy as np
import ml_dtypes
import bass_rust
import concourse.bass as bass
import concourse.mybir as mybir
from concourse.tile import TileContext, add_dep_helper
from concourse.masks import make_identity

B, L, D, H = 2, 2048, 2048, 16
HD = 128
N_CORES = 8
HPC = H // N_CORES          # heads per core = 2
ES = HPC * HD               # 256 = e-slice width per core
T = B * L                   # 4096 tokens total
P = 128
SCALE = 1.0 / float(np.sqrt(HD))
NEG = -30000.0              # causal mask fill; exp(SCALE*(s+NEG)) underflows to 0
FP = mybir.dt.float32
BF = mybir.dt.bfloat16

N_TT = T // P               # 32 global t-tiles
N_LT = L // P               # 16 t-tiles per batch
N_DT = D // P               # 16 d-tiles
NQC = 4                     # 512-token q-chunks per batch

# attention-out AllGather pieces per batch, in units of 512-t q-chunks
AG_PIECES = {0: [(0, 2), (2, 4)], 1: [(0, 2), (2, 3), (3, 4)]}

# phase-2 block geometry: for (kt, qc) with qc >= kt//4:
#   off  = max(0, kt*128 - qc*512)   (column offset within the qc chunk)
#   w    = 512 - off
# blocks laid out kt-major in the expST tile
BLOCKS = []          # (kt, qc, off, w, boff)
_boff = 0
for _kt in range(16):
    for _qc in range(_kt // 4, 4):
        _off = max(0, _kt * 128 - _qc * 512)
        _w = 512 - _off
        BLOCKS.append((_kt, _qc, _off, _w, _boff))
        _boff += _w
EXP_COLS = _boff     # 17408
BLK = {(kt, qc): (off, w, boff) for (kt, qc, off, w, boff) in BLOCKS}


def split_multi_waits(nc):
    """This walrus build allows 1 sync wait per instruction (2 for
    EventSemaphore). Tile attaches more on some instructions (tail drain,
    collective-adjacent DMAs); hoist the extras onto same-engine NoOps."""
    for f in nc.m.functions:
        for bb in f.blocks:
            new_insts = []
            changed = False
            for ins in bb.instructions:
                si = ins.sync_info
                cap = 2 if type(ins).__name__ == "InstEventSemaphore" else 1
                if si is not None and len(si.on_wait) > cap:
                    waits = list(si.on_wait)
                    for k, w in enumerate(waits[cap:]):
                        new_insts.append(mybir.InstNoOp(
                            name=f"{ins.name}-wsplit{k}", ins=[], outs=[],
                            engine=ins.engine,
                            sync_info=bass_rust.SyncInfo(on_wait=[w], on_update=[]),
                        ))
                    ins.sync_info = bass_rust.SyncInfo(
                        on_wait=waits[:cap], on_update=list(si.on_update))
                    changed = True
                new_insts.append(ins)
            if changed:
                bb.instructions.clear()
                for i2 in new_insts:
                    bb.add_instruction(i2)


def make_causal_mask_T(nc, ap, mask_val):
    """mask[k, q] = 0 if k <= q else mask_val (transposed causal)."""
    sq = ap.shape[0]
    nc.gpsimd.memset(ap, 0.0)
    nc.gpsimd.affine_select(
        out=ap, in_=ap,
        compare_op=mybir.AluOpType.is_ge,
        fill=mask_val, base=0,
        # keep where (-x + y) >= 0, i.e. k <= q
        pattern=[[1, sq]],
        channel_multiplier=-1,
    )


def build(debug=False, fix_waits=True):
    nc = bass.Bass()
    xT = nc.declare_dram_parameter("xT", [D, T], BF, isOutput=False)
    wqkvT = nc.declare_dram_parameter("wqkvT", [D, 3 * ES], BF, isOutput=False)
    # per-head stacked trig tables: rows 0:64 = even-col table, 64:128 = odd
    cc_p = [nc.declare_dram_parameter(f"cc{h}", [P, L], BF, isOutput=False)
            for h in range(HPC)]
    ss_p = [nc.declare_dram_parameter(f"ss{h}", [P, L], BF, isOutput=False)
            for h in range(HPC)]
    woT = nc.declare_dram_parameter("woT", [D, ES], BF, isOutput=False)
    out = nc.declare_dram_parameter("out", [ES, T], FP, isOutput=True)
    if debug:
        dbg_qt = nc.declare_dram_parameter("dbg_qt", [P, HPC * T], FP, isOutput=True)
        dbg_kt = nc.declare_dram_parameter("dbg_kt", [P, HPC * T], FP, isOutput=True)
        dbg_v = nc.declare_dram_parameter("dbg_v", [P, N_TT * ES], FP, isOutput=True)
        dbg_ob = nc.declare_dram_parameter("dbg_ob", [P, B * HPC * L], FP,
                                           isOutput=True)

    o_bounce, ag_o = {}, {}
    for b, pieces in AG_PIECES.items():
        for (c0, c1) in pieces:
            w = (c1 - c0) * 512
            o_bounce[(b, c0)] = nc.dram_tensor(f"o_bounce{b}_{c0}", [ES, w], BF)
            ag_o[(b, c0)] = nc.dram_tensor(f"ag_o{b}_{c0}", [N_CORES * ES, w], BF,
                                           addr_space="Shared")
    rg = [list(range(N_CORES))]

    with TileContext(nc, pool_alloc_mode="queue") as tc:
        with (
            tc.tile_pool(name="const", bufs=1) as const_pool,
            tc.tile_pool(name="resident", bufs=1) as res_pool,
            tc.tile_pool(name="wo", bufs=1) as wo_pool,
            tc.tile_pool(name="vt", bufs=1) as vt_pool,
        ):
            ident = const_pool.tile([P, P], BF, name="ident")
            make_identity(nc, ident[:, :])
            cmT = const_pool.tile([P, P], FP, name="cmT")
            make_causal_mask_T(nc, cmT[:, :], NEG)
            ones = const_pool.tile([P, P], BF, name="ones")
            nc.gpsimd.memset(ones[:, :], 1.0)

            # resident through phases 1-2
            qt_sb = res_pool.tile([P, HPC * T], BF, name="qt_sb")   # [hd', h*T+t]
            kt_sb = res_pool.tile([P, HPC * T], BF, name="kt_sb")
            v_sb = res_pool.tile([P, N_TT * ES], BF, name="v_sb")   # [t%128, tt*ES+e]
            woT_sb = wo_pool.tile([P, N_DT * ES], BF, name="woT_sb")

            # ---------------- phase 1: transposed QKV + RoPE ----------------
            # eb order: q-h0, q-h1, k-h0, k-h1, v-0, v-1
            with (
                tc.tile_pool(name="wq", bufs=1) as wq_pool,
                tc.tile_pool(name="xt", bufs=1) as xt_pool,
                tc.tile_pool(name="rsc", bufs=1) as rsc_pool,
                tc.tile_pool(name="psG", bufs=2, space="PSUM") as psG,
            ):
                wt_sb = wq_pool.tile([P, N_DT * 3 * ES], BF, name="wt_sb")
                cc_sb = [wq_pool.tile([P, L], BF, name=f"cc{h}_sb")
                         for h in range(HPC)]
                ss_sb = [wq_pool.tile([P, L], BF, name=f"ss{h}_sb")
                         for h in range(HPC)]
                xt_sb = xt_pool.tile([P, N_DT * 2048], BF, name="xt_sb")
                vt_sb = vt_pool.tile([P, HPC * T], BF,
                                     name="vt_sb")  # [e, eb*T + t]

                # DMA priority: x^T th0 tiles + weights interleaved, then trig
                for dt in range(N_DT):
                    nc.sync.dma_start(
                        out=xt_sb[:, dt * 2048:(dt + 1) * 2048],
                        in_=xT[dt * P:(dt + 1) * P, 0:2048])
                    nc.sync.dma_start(
                        out=wt_sb[:, dt * 3 * ES:(dt + 1) * 3 * ES],
                        in_=wqkvT[dt * P:(dt + 1) * P, :])
                for h in range(HPC):
                    nc.sync.dma_start(out=cc_sb[h][:, :], in_=cc_p[h][:, :])
                    nc.sync.dma_start(out=ss_sb[h][:, :], in_=ss_p[h][:, :])

                def rope_drain(gp, dst, h, th):
                    cc, ss = cc_sb[h], ss_sb[h]
                    dcol = slice(h * T + th * 2048, h * T + (th + 1) * 2048)
                    e_ps, o_ps = gp[0:64, :], gp[64:128, :]
                    t1 = rsc_pool.tile([64, 2048], FP, name="t1", tag="t1")
                    t2 = rsc_pool.tile([64, 2048], FP, name="t2", tag="t2")
                    nc.vector.tensor_tensor(t1[:, :], e_ps, cc[0:64, :],
                                            op=mybir.AluOpType.mult)
                    nc.vector.tensor_tensor(t2[:, :], o_ps, ss[0:64, :],
                                            op=mybir.AluOpType.mult)
                    nc.vector.tensor_tensor(dst[0:64, dcol], t1[:, :], t2[:, :],
                                            op=mybir.AluOpType.subtract)
                    t3 = rsc_pool.tile([64, 2048], FP, name="t3", tag="t1")
                    t4 = rsc_pool.tile([64, 2048], FP, name="t4", tag="t2")
                    nc.vector.tensor_tensor(t3[:, :], o_ps, cc[64:128, :],
                                            op=mybir.AluOpType.mult)
                    nc.vector.tensor_tensor(t4[:, :], e_ps, ss[64:128, :],
                                            op=mybir.AluOpType.mult)
                    nc.vector.tensor_tensor(dst[64:128, dcol], t3[:, :], t4[:, :],
                                            op=mybir.AluOpType.add)

                for th in range(2):
                    for ebi in range(6):
                        gp = psG.tile([P, 2048], FP, name="gp", tag="gp")
                        for dt in range(N_DT):
                            lhsT = wt_sb[:, dt * 3 * ES + ebi * P:
                                         dt * 3 * ES + (ebi + 1) * P]
                            for c in range(4):
                                nc.tensor.matmul(
                                    gp[:, c * 512:(c + 1) * 512], lhsT,
                                    xt_sb[:, dt * 2048 + c * 512:
                                          dt * 2048 + (c + 1) * 512],
                                    start=(dt == 0), stop=(dt == N_DT - 1))
                            if th == 0 and ebi == 5:
                                nc.sync.dma_start(
                                    out=xt_sb[:, dt * 2048:(dt + 1) * 2048],
                                    in_=xT[dt * P:(dt + 1) * P, 2048:4096])
                        if ebi < 2:
                            rope_drain(gp, qt_sb, ebi, th)
                        elif ebi < 4:
                            rope_drain(gp, kt_sb, ebi - 2, th)
                        else:
                            eb2 = ebi - 4
                            nc.scalar.copy(
                                vt_sb[:, eb2 * T + th * 2048:
                                      eb2 * T + (th + 1) * 2048],
                                gp[:, :])

            # v^T -> v transposes (own PSUM scope, after psG frees)
            with tc.tile_pool(name="psT", bufs=2, space="PSUM") as psT:
                for eb2 in range(2):
                    for tg in range(8):  # groups of 4 of the 32 t-tiles
                        tr = psT.tile([P, 512], BF, name="tr", tag="tr")
                        for j in range(4):
                            tt_g = tg * 4 + j
                            nc.tensor.transpose(
                                tr[:, j * P:(j + 1) * P],
                                vt_sb[:, eb2 * T + tt_g * P:
                                      eb2 * T + (tt_g + 1) * P],
                                ident[:, :])
                        for j in range(4):
                            tt_g = tg * 4 + j
                            nc.vector.tensor_copy(
                                v_sb[:, tt_g * ES + eb2 * P:
                                     tt_g * ES + (eb2 + 1) * P],
                                tr[:, j * P:(j + 1) * P])

            if debug:
                with tc.tile_pool(name="dbgp", bufs=2) as dbgp:
                    for nm, src, dd in (("q", qt_sb, dbg_qt), ("k", kt_sb, dbg_kt),
                                        ("v", v_sb, dbg_v)):
                        for i in range(HPC * T // 512):
                            s = dbgp.tile([P, 512], FP, name="dstage")
                            nc.vector.tensor_copy(s[:, :],
                                                  src[:, i * 512:(i + 1) * 512])
                            nc.sync.dma_start(out=dd[:, i * 512:(i + 1) * 512],
                                              in_=s[:, :])

            # ---------------- phases 2+3 (interleaved) ----------------
            for dt in range(N_DT):
                nc.sync.dma_start(out=woT_sb[:, dt * ES:(dt + 1) * ES],
                                  in_=woT[dt * P:(dt + 1) * P, :])

            with (
                tc.tile_pool(name="pexp", bufs=2) as pexp,
                tc.tile_pool(name="prec", bufs=2) as prec,
                tc.tile_pool(name="p2ob", bufs=2) as p2ob,
                tc.tile_pool(name="p3x", bufs=2) as p3x,
                tc.tile_pool(name="p3o", bufs=4) as p3o,
                tc.tile_pool(name="psS", bufs=3, space="PSUM") as psS,
                tc.tile_pool(name="psSum", bufs=1, space="PSUM") as psSum,
                tc.tile_pool(name="psO", bufs=2, space="PSUM") as psO,
                tc.tile_pool(name="psW", bufs=2, space="PSUM") as psW,
            ):
                ob_tiles = {}
                pv_done = {}

                def scores_group(b, h, S, ktg):
                    """score blocks for k-tiles ktg*4..ktg*4+3 (kt-major)."""
                    qoff = h * T + b * L
                    for kt in range(ktg * 4, ktg * 4 + 4):
                        lhsT = kt_sb[:, qoff + kt * P: qoff + (kt + 1) * P]
                        for qc in range(kt // 4, 4):
                            off, w, boff = BLK[(kt, qc)]
                            sp = psS.tile([P, 512], FP, name="sp", tag="sp")
                            nc.tensor.matmul(
                                sp[:, :w], lhsT,
                                qt_sb[:, qoff + qc * 512 + off:
                                      qoff + (qc + 1) * 512],
                                start=True, stop=True)
                            if qc == kt // 4:  # leading 128 cols are diagonal
                                nc.vector.tensor_tensor(
                                    sp[:, 0:P], sp[:, 0:P], cmT[:, :],
                                    op=mybir.AluOpType.add)
                            nc.scalar.activation(
                                S[:, boff:boff + w], sp[:, :w],
                                mybir.ActivationFunctionType.Exp, scale=SCALE)

                def pv_chunk(b, h, qc, S, ob_sb):
                    """softmax-normalize + PV for one 512-q chunk."""
                    nkt = 4 * qc + 4
                    sm = psSum.tile([P, 512], FP, name="sm", tag="sm")
                    for kt in range(nkt):
                        off, w, boff = BLK[(kt, qc)]
                        nc.tensor.matmul(sm[:, off:], ones[:, :],
                                         S[:, boff:boff + w],
                                         start=(kt == 0), stop=(kt == nkt - 1))
                    lsm = prec.tile([P, 512], FP, name="lsm", tag="lsm")
                    nc.scalar.activation(lsm[:, :], sm[:, :],
                                         mybir.ActivationFunctionType.Ln)
                    rec = prec.tile([P, 512], FP, name="rec", tag="rec")
                    nc.scalar.activation(rec[:, :], lsm[:, :],
                                         mybir.ActivationFunctionType.Exp,
                                         scale=-1.0)
                    o_ps = psO.tile([P, 512], FP, name="o_ps", tag="o")
                    for kt in range(nkt):
                        off, w, boff = BLK[(kt, qc)]
                        nc.tensor.matmul(
                            o_ps[:, off:],
                            v_sb[:, (b * N_LT + kt) * ES + h * HD:
                                 (b * N_LT + kt) * ES + (h + 1) * HD],
                            S[:, boff:boff + w],
                            start=(kt == 0), stop=(kt == nkt - 1))
                    obcp = nc.vector.tensor_tensor(
                        ob_sb[:, h * L + qc * 512:h * L + (qc + 1) * 512],
                        o_ps[:, :], rec[:, :], op=mybir.AluOpType.mult)
                    pv_done[(b, h, qc)] = obcp

                def ag_fire(b, c0, c1, ob_sb):
                    for h in range(HPC):
                        nc.sync.dma_start(
                            out=o_bounce[(b, c0)][h * HD:(h + 1) * HD, :],
                            in_=ob_sb[:, h * L + c0 * 512:h * L + c1 * 512])
                    nc.gpsimd.collective_compute(
                        "AllGather", mybir.AluOpType.bypass,
                        ins=[o_bounce[(b, c0)][:]],
                        outs=[ag_o[(b, c0)][:]],
                        replica_groups=rg)


                def phase3(b, c0, c1):
                    w = (c1 - c0) * 512
                    nch = w // 512
                    ots = []
                    for tch in range(nch):
                        ot = p3x.tile([P, N_DT * 512], BF, name="ot", tag="ot")
                        for dt in range(N_DT):
                            nc.sync.dma_start(
                                out=ot[:, dt * 512:(dt + 1) * 512],
                                in_=ag_o[(b, c0)][dt * P:(dt + 1) * P,
                                                  tch * 512:(tch + 1) * 512])
                        ots.append(ot)
                    # same-weight pairing across the piece's chunks
                    for et in range(HPC):
                        fps = [psW.tile([P, 512], FP, name="f_ps", tag="f")
                               for _ in range(nch)]
                        for dt in range(N_DT):
                            lhsT = woT_sb[:, dt * ES + et * P:
                                          dt * ES + (et + 1) * P]
                            for i in range(nch):
                                nc.tensor.matmul(
                                    fps[i][:, :], lhsT,
                                    ots[i][:, dt * 512:(dt + 1) * 512],
                                    start=(dt == 0), stop=(dt == N_DT - 1))
                        for i in range(nch):
                            t0 = b * L + (c0 + i) * 512
                            f_sb = p3o.tile([P, 512], FP, name="f_sb")
                            nc.vector.tensor_copy(f_sb[:, :], fps[i][:, :])
                            nc.sync.dma_start(
                                out=out[et * P:(et + 1) * P, t0:t0 + 512],
                                in_=f_sb[:, :])

                ph3_after = {(1, 0): [(0, 0, 2)], (1, 1): [(0, 2, 4)],
                             (1, 2): [(1, 0, 2)],
                             (1, 3): [(1, 2, 3), (1, 3, 4)]}
                for b in range(B):
                    Ss = [pexp.tile([P, EXP_COLS], BF, name=f"S{h}", tag="S")
                          for h in range(HPC)]
                    ob_tiles[b] = p2ob.tile([P, HPC * L], BF,
                                            name="ob_sb", tag="ob")
                    ob_sb = ob_tiles[b]
                    for qc in range(4):
                        for h in range(HPC):
                            scores_group(b, h, Ss[h], qc)
                        for h in range(HPC):
                            pv_chunk(b, h, qc, Ss[h], ob_sb)
                        for (c0, c1) in AG_PIECES[b]:
                            if c1 == qc + 1:
                                ag_fire(b, c0, c1, ob_sb)
                        for args in ph3_after.get((b, qc), []):
                            phase3(*args)

                if debug:
                    with tc.tile_pool(name="dbgo", bufs=2) as dbgo:
                        for b in range(B):
                            for i in range(HPC * L // 512):
                                s = dbgo.tile([P, 512], FP, name="dob")
                                nc.vector.tensor_copy(
                                    s[:, :],
                                    ob_tiles[b][:, i * 512:(i + 1) * 512])
                                nc.sync.dma_start(
                                    out=dbg_ob[:, b * HPC * L + i * 512:
                                               b * HPC * L + (i + 1) * 512],
                                    in_=s[:, :])

    if fix_waits:
        split_multi_waits(nc)
    return nc


def make_in_maps(x, cos, sin, Wqkv, Wo):
    bf = ml_dtypes.bfloat16
    xT_full = np.ascontiguousarray(
        np.asarray(x).reshape(T, D).T).astype(bf)
    # q/k row permutation: head-major, evens then odds
    perm = []
    for h in range(HPC):
        perm.extend(h * HD + 2 * np.arange(64))
        perm.extend(h * HD + 2 * np.arange(64) + 1)
    perm = np.asarray(perm)
    in_maps = []
    cosA, sinA = np.asarray(cos), np.asarray(sin)
    for c in range(N_CORES):
        cols = slice(c * ES, (c + 1) * ES)
        wq = Wqkv[c * ES:(c + 1) * ES, :][perm]
        wk = Wqkv[D + c * ES: D + (c + 1) * ES, :][perm]
        wv = Wqkv[2 * D + c * ES: 2 * D + (c + 1) * ES, :]
        w_c = np.concatenate([wq, wk, wv], axis=0)
        m = {
            "xT": xT_full,
            "wqkvT": np.ascontiguousarray(w_c.T.astype(bf)),
            "woT": np.ascontiguousarray(Wo[cols, :].T.astype(bf)),
        }
        for h in range(HPC):
            base = c * ES + h * HD
            ce = cosA[:, base + 2 * np.arange(64)].T      # [64, L]
            co = cosA[:, base + 2 * np.arange(64) + 1].T
            se = sinA[:, base + 2 * np.arange(64)].T
            so = sinA[:, base + 2 * np.arange(64) + 1].T
            m[f"cc{h}"] = np.ascontiguousarray(
                np.concatenate([ce, co], axis=0)).astype(bf)
            m[f"ss{h}"] = np.ascontiguousarray(
                np.concatenate([se, so], axis=0)).astype(bf)
        in_maps.append(m)
    return in_maps


_cache = {}


def kernel(x, cos, sin, Wqkv, Wo):
    from concourse.bass_utils import run_bass_kernel_spmd
    x = np.asarray(x, dtype=np.float32)
    cos = np.asarray(cos, dtype=np.float32)
    sin = np.asarray(sin, dtype=np.float32)
    Wqkv = np.asarray(Wqkv, dtype=np.float32)
    Wo = np.asarray(Wo, dtype=np.float32)
    if "nc" not in _cache:
        _cache["nc"] = build()
    nc = _cache["nc"]
    in_maps = make_in_maps(x, cos, sin, Wqkv, Wo)
    res = run_bass_kernel_spmd(nc, in_maps, core_ids=list(range(N_CORES)))
    pieces = [res.results[c]["out"].T for c in range(N_CORES)]
    return np.concatenate(pieces, axis=1).reshape(B, L, D)


# revision 17
# speedup vs baseline: 1.0979x; 1.0089x over previous
"""Distributed causal attention block (QKV + RoPE + SDPA + Wo) on 8 TRN2 cores.

Sharding: tensor-parallel over heads (2 heads/core). Each core:
  phase 1: weight-stationary transposed QKV: q^T/k^T/v^T = Wqkv_c @ x^T
           streamed from host-pretransposed x^T (bf16); RoPE applied in the
           [e, t] layout with per-head even/odd partition split; v^T is
           PE-transposed back to [t, e] for the PV matmuls
  phase 2: causal attention per (batch, head) with TRANSPOSED scores
           s^T[k, q] (no P transposes); softmax sums via all-ones matmul
           (broadcast across partitions); 1/sum folded into the o^T drain
  phase 3: AllGather attention outputs (progressive pieces) -> Wo e-slice,
           emitted interleaved with phase 2 to avoid a serial tail
Host concatenates the 8 e-slices.

The q/k rows of Wqkv (and cos/sin tables) are permuted head-major
even/odd on the host; attention scores are invariant to a shared
permutation of the head dim of Q and K.
"""
import numpy as np
import ml_dtypes
import bass_rust
import concourse.bass as bass
import concourse.mybir as mybir
from concourse.tile import TileContext, add_dep_helper
from concourse.masks import make_identity

B, L, D, H = 2, 2048, 2048, 16
HD = 128
N_CORES = 8
HPC = H // N_CORES          # heads per core = 2
ES = HPC * HD               # 256 = e-slice width per core
T = B * L                   # 4096 tokens total
P = 128
SCALE = 1.0 / float(np.sqrt(HD))
NEG = -30000.0              # causal mask fill; exp(SCALE*(s+NEG)) underflows to 0
FP = mybir.dt.float32
BF = mybir.dt.bfloat16

N_TT = T // P               # 32 global t-tiles
N_LT = L // P               # 16 t-tiles per batch
N_DT = D // P               # 16 d-tiles
NQC = 4                     # 512-token q-chunks per batch

# attention-out AllGather pieces per batch, in units of 512-t q-chunks
AG_PIECES = {0: [(0, 2), (2, 4)], 1: [(0, 2), (2, 3), (3, 4)]}

# phase-2 block geometry: for (kt, qc) with qc >= kt//4:
#   off  = max(0, kt*128 - qc*512)   (column offset within the qc chunk)
#   w    = 512 - off
# blocks laid out kt-major in the expST tile
BLOCKS = []          # (kt, qc, off, w, boff)
_boff = 0
for _kt in range(16):
    for _qc in range(_kt // 4, 4):
        _off = max(0, _kt * 128 - _qc * 512)
        _w = 512 - _off
        BLOCKS.append((_kt, _qc, _off, _w, _boff))
        _boff += _w
EXP_COLS = _boff     # 17408
BLK = {(kt, qc): (off, w, boff) for (kt, qc, off, w, boff) in BLOCKS}


def split_multi_waits(nc):
    """This walrus build allows 1 sync wait per instruction (2 for
    EventSemaphore). Tile attaches more on some instructions (tail drain,
    collective-adjacent DMAs); hoist the extras onto same-engine NoOps."""
    for f in nc.m.functions:
        for bb in f.blocks:
            new_insts = []
            changed = False
            for ins in bb.instructions:
                si = ins.sync_info
                cap = 2 if type(ins).__name__ == "InstEventSemaphore" else 1
                if si is not None and len(si.on_wait) > cap:
                    waits = list(si.on_wait)
                    for k, w in enumerate(waits[cap:]):
                        new_insts.append(mybir.InstNoOp(
                            name=f"{ins.name}-wsplit{k}", ins=[], outs=[],
                            engine=ins.engine,
                            sync_info=bass_rust.SyncInfo(on_wait=[w], on_update=[]),
                        ))
                    ins.sync_info = bass_rust.SyncInfo(
                        on_wait=waits[:cap], on_update=list(si.on_update))
                    changed = True
                new_insts.append(ins)
            if changed:
                bb.instructions.clear()
                for i2 in new_insts:
                    bb.add_instruction(i2)


def make_causal_mask_T(nc, ap, mask_val):
    """mask[k, q] = 0 if k <= q else mask_val (transposed causal)."""
    sq = ap.shape[0]
    nc.gpsimd.memset(ap, 0.0)
    nc.gpsimd.affine_select(
        out=ap, in_=ap,
        compare_op=mybir.AluOpType.is_ge,
        fill=mask_val, base=0,
        # keep where (-x + y) >= 0, i.e. k <= q
        pattern=[[1, sq]],
        channel_multiplier=-1,
    )


def build(debug=False, fix_waits=True):
    nc = bass.Bass()
    xT = nc.declare_dram_parameter("xT", [D, T], BF, isOutput=False)
    wqkvT = nc.declare_dram_parameter("wqkvT", [D, 3 * ES], BF, isOutput=False)
    # per-head stacked trig tables: rows 0:64 = even-col table, 64:128 = odd
    cc_p = [nc.declare_dram_parameter(f"cc{h}", [P, L], BF, isOutput=False)
            for h in range(HPC)]
    ss_p = [nc.declare_dram_parameter(f"ss{h}", [P, L], BF, isOutput=False)
            for h in range(HPC)]
    woT = nc.declare_dram_parameter("woT", [D, ES], BF, isOutput=False)
    out = nc.declare_dram_parameter("out", [ES, T], FP, isOutput=True)
    if debug:
        dbg_qt = nc.declare_dram_parameter("dbg_qt", [P, HPC * T], FP, isOutput=True)
        dbg_kt = nc.declare_dram_parameter("dbg_kt", [P, HPC * T], FP, isOutput=True)
        dbg_v = nc.declare_dram_parameter("dbg_v", [P, N_TT * ES], FP, isOutput=True)
        dbg_ob = nc.declare_dram_parameter("dbg_ob", [P, B * HPC * L], FP,
                                           isOutput=True)

    o_bounce, ag_o = {}, {}
    for b, pieces in AG_PIECES.items():
        for (c0, c1) in pieces:
            w = (c1 - c0) * 512
            o_bounce[(b, c0)] = nc.dram_tensor(f"o_bounce{b}_{c0}", [ES, w], BF)
            ag_o[(b, c0)] = nc.dram_tensor(f"ag_o{b}_{c0}", [N_CORES * ES, w], BF,
                                           addr_space="Shared")
    rg = [list(range(N_CORES))]

    with TileContext(nc, pool_alloc_mode="queue") as tc:
        with (
            tc.tile_pool(name="const", bufs=1) as const_pool,
            tc.tile_pool(name="resident", bufs=1) as res_pool,
            tc.tile_pool(name="wo", bufs=1) as wo_pool,
            tc.tile_pool(name="vt", bufs=1) as vt_pool,
        ):
            ident = const_pool.tile([P, P], BF, name="ident")
            make_identity(nc, ident[:, :])
            ones = const_pool.tile([P, P], BF, name="ones")
            nc.gpsimd.memset(ones[:, :], 1.0)

            # resident through phases 1-2
            qt_sb = res_pool.tile([P, HPC * T], BF, name="qt_sb")   # [hd', h*T+t]
            kt_sb = res_pool.tile([P, HPC * T], BF, name="kt_sb")
            v_sb = res_pool.tile([P, N_TT * ES], BF, name="v_sb")   # [t%128, tt*ES+e]
            woT_sb = wo_pool.tile([P, N_DT * ES], BF, name="woT_sb")

            # ---------------- phase 1: transposed QKV + RoPE ----------------
            # eb order: q-h0, q-h1, k-h0, k-h1, v-0, v-1
            with (
                tc.tile_pool(name="wq", bufs=1) as wq_pool,
                tc.tile_pool(name="xt", bufs=1) as xt_pool,
                tc.tile_pool(name="rsc", bufs=1) as rsc_pool,
                tc.tile_pool(name="psG", bufs=2, space="PSUM") as psG,
            ):
                wt_sb = wq_pool.tile([P, N_DT * 3 * ES], BF, name="wt_sb")
                cc_sb = [wq_pool.tile([P, L], BF, name=f"cc{h}_sb")
                         for h in range(HPC)]
                ss_sb = [wq_pool.tile([P, L], BF, name=f"ss{h}_sb")
                         for h in range(HPC)]
                xt_sb = xt_pool.tile([P, N_DT * 2048], BF, name="xt_sb")
                vt_sb = vt_pool.tile([P, HPC * T], BF,
                                     name="vt_sb")  # [e, eb*T + t]

                # DMA priority: x^T th0 tiles + weights interleaved, then trig
                for dt in range(N_DT):
                    nc.sync.dma_start(
                        out=xt_sb[:, dt * 2048:(dt + 1) * 2048],
                        in_=xT[dt * P:(dt + 1) * P, 0:2048])
                    nc.sync.dma_start(
                        out=wt_sb[:, dt * 3 * ES:(dt + 1) * 3 * ES],
                        in_=wqkvT[dt * P:(dt + 1) * P, :])
                for h in range(HPC):
                    nc.sync.dma_start(out=cc_sb[h][:, :], in_=cc_p[h][:, :])
                    nc.sync.dma_start(out=ss_sb[h][:, :], in_=ss_p[h][:, :])

                def rope_drain(gp, dst, h, th):
                    cc, ss = cc_sb[h], ss_sb[h]
                    dcol = slice(h * T + th * 2048, h * T + (th + 1) * 2048)
                    e_ps, o_ps = gp[0:64, :], gp[64:128, :]
                    t1 = rsc_pool.tile([64, 2048], FP, name="t1", tag="t1")
                    t2 = rsc_pool.tile([64, 2048], FP, name="t2", tag="t2")
                    nc.vector.tensor_tensor(t1[:, :], e_ps, cc[0:64, :],
                                            op=mybir.AluOpType.mult)
                    nc.vector.tensor_tensor(t2[:, :], o_ps, ss[0:64, :],
                                            op=mybir.AluOpType.mult)
                    nc.vector.tensor_tensor(dst[0:64, dcol], t1[:, :], t2[:, :],
                                            op=mybir.AluOpType.subtract)
                    t3 = rsc_pool.tile([64, 2048], FP, name="t3", tag="t1")
                    t4 = rsc_pool.tile([64, 2048], FP, name="t4", tag="t2")
                    nc.vector.tensor_tensor(t3[:, :], o_ps, cc[64:128, :],
                                            op=mybir.AluOpType.mult)
                    nc.vector.tensor_tensor(t4[:, :], e_ps, ss[64:128, :],
                                            op=mybir.AluOpType.mult)
                    nc.vector.tensor_tensor(dst[64:128, dcol], t3[:, :], t4[:, :],
                                            op=mybir.AluOpType.add)

                EB_ORDER = [4, 5, 0, 1, 2, 3]   # v first; tables can lag
                for th in range(2):
                    for i, ebi in enumerate(EB_ORDER):
                        gp = psG.tile([P, 2048], FP, name="gp", tag="gp")
                        for dt in range(N_DT):
                            lhsT = wt_sb[:, dt * 3 * ES + ebi * P:
                                         dt * 3 * ES + (ebi + 1) * P]
                            for c in range(4):
                                nc.tensor.matmul(
                                    gp[:, c * 512:(c + 1) * 512], lhsT,
                                    xt_sb[:, dt * 2048 + c * 512:
                                          dt * 2048 + (c + 1) * 512],
                                    start=(dt == 0), stop=(dt == N_DT - 1))
                            if th == 0 and i == 5:
                                nc.sync.dma_start(
                                    out=xt_sb[:, dt * 2048:(dt + 1) * 2048],
                                    in_=xT[dt * P:(dt + 1) * P, 2048:4096])
                        if ebi < 2:
                            rope_drain(gp, qt_sb, ebi, th)
                        elif ebi < 4:
                            rope_drain(gp, kt_sb, ebi - 2, th)
                        else:
                            eb2 = ebi - 4
                            nc.scalar.copy(
                                vt_sb[:, eb2 * T + th * 2048:
                                      eb2 * T + (th + 1) * 2048],
                                gp[:, :])

            # ---------------- phases 2+3 (interleaved) ----------------
            for dt in range(N_DT):
                nc.sync.dma_start(out=woT_sb[:, dt * ES:(dt + 1) * ES],
                                  in_=woT[dt * P:(dt + 1) * P, :])

            with (
                tc.tile_pool(name="pexp", bufs=2) as pexp,
                tc.tile_pool(name="prec", bufs=2) as prec,
                tc.tile_pool(name="p2ob", bufs=2) as p2ob,
                tc.tile_pool(name="p3x", bufs=2) as p3x,
                tc.tile_pool(name="p3o", bufs=2) as p3o,
                tc.tile_pool(name="psS", bufs=3, space="PSUM") as psS,
                tc.tile_pool(name="psSum", bufs=1, space="PSUM") as psSum,
                tc.tile_pool(name="psO", bufs=2, space="PSUM") as psO,
            ):
                ob_tiles = {}

                def scores_group(b, h, S, ktg):
                    """score blocks for k-tiles ktg*4..ktg*4+3 (kt-major)."""
                    qoff = h * T + b * L
                    for kt in range(ktg * 4, ktg * 4 + 4):
                        lhsT = kt_sb[:, qoff + kt * P: qoff + (kt + 1) * P]
                        for qc in range(kt // 4, 4):
                            off, w, boff = BLK[(kt, qc)]
                            sp = psS.tile([P, 512], FP, name="sp", tag="sp")
                            nc.tensor.matmul(
                                sp[:, :w], lhsT,
                                qt_sb[:, qoff + qc * 512 + off:
                                      qoff + (qc + 1) * 512],
                                start=True, stop=True)
                            nc.scalar.activation(
                                S[:, boff:boff + w], sp[:, :w],
                                mybir.ActivationFunctionType.Exp, scale=SCALE)
                            if qc == kt // 4:  # zero masked (k>q) triangle
                                nc.gpsimd.affine_select(
                                    out=S[:, boff:boff + P],
                                    in_=S[:, boff:boff + P],
                                    compare_op=mybir.AluOpType.is_ge,
                                    fill=0.0, base=0,
                                    pattern=[[1, P]], channel_multiplier=-1)

                def pv_chunk(b, h, qc, S, ob_sb):
                    """softmax-normalize + PV for one 512-q chunk."""
                    nkt = 4 * qc + 4
                    # k-block partial sums accumulated on DVE (fp32), then a
                    # single all-ones matmul for the cross-partition reduce
                    fulls = [kt for kt in range(nkt) if BLK[(kt, qc)][0] == 0]
                    parts = [kt for kt in range(nkt) if BLK[(kt, qc)][0] > 0]
                    acc = prec.tile([P, 512], FP, name="acc", tag="acc")
                    if len(fulls) == 1:
                        bo = BLK[(fulls[0], qc)][2]
                        nc.vector.tensor_copy(acc[:, :], S[:, bo:bo + 512])
                    else:
                        bo0, bo1 = (BLK[(fulls[0], qc)][2],
                                    BLK[(fulls[1], qc)][2])
                        nc.vector.tensor_tensor(
                            acc[:, :], S[:, bo0:bo0 + 512],
                            S[:, bo1:bo1 + 512], op=mybir.AluOpType.add)
                        for kt in fulls[2:]:
                            bo = BLK[(kt, qc)][2]
                            nc.vector.tensor_tensor(
                                acc[:, :], acc[:, :], S[:, bo:bo + 512],
                                op=mybir.AluOpType.add)
                    for kt in parts:
                        off, w, bo = BLK[(kt, qc)]
                        nc.vector.tensor_tensor(
                            acc[:, off:], acc[:, off:], S[:, bo:bo + w],
                            op=mybir.AluOpType.add)
                    accb = prec.tile([P, 512], BF, name="accb", tag="accb")
                    nc.vector.tensor_copy(accb[:, :], acc[:, :])
                    sm = psSum.tile([P, 512], FP, name="sm", tag="sm")
                    nc.tensor.matmul(sm[:, :], ones[:, :], accb[:, :],
                                     start=True, stop=True)
                    lsm = prec.tile([P, 512], FP, name="lsm", tag="lsm")
                    nc.scalar.activation(lsm[:, :], sm[:, :],
                                         mybir.ActivationFunctionType.Ln)
                    rec = prec.tile([P, 512], FP, name="rec", tag="rec")
                    nc.scalar.activation(rec[:, :], lsm[:, :],
                                         mybir.ActivationFunctionType.Exp,
                                         scale=-1.0)
                    o_ps = psO.tile([P, 512], FP, name="o_ps", tag="o")
                    for kt in range(nkt):
                        off, w, boff = BLK[(kt, qc)]
                        nc.tensor.matmul(
                            o_ps[:, off:],
                            v_sb[:, (b * N_LT + kt) * ES + h * HD:
                                 (b * N_LT + kt) * ES + (h + 1) * HD],
                            S[:, boff:boff + w],
                            start=(kt == 0), stop=(kt == nkt - 1))
                    nc.vector.tensor_tensor(
                        ob_sb[:, h * L + qc * 512:h * L + (qc + 1) * 512],
                        o_ps[:, :], rec[:, :], op=mybir.AluOpType.mult)

                def ag_fire(b, c0, c1, ob_sb):
                    for h in range(HPC):
                        nc.sync.dma_start(
                            out=o_bounce[(b, c0)][h * HD:(h + 1) * HD, :],
                            in_=ob_sb[:, h * L + c0 * 512:h * L + c1 * 512])
                    nc.gpsimd.collective_compute(
                        "AllGather", mybir.AluOpType.bypass,
                        ins=[o_bounce[(b, c0)][:]],
                        outs=[ag_o[(b, c0)][:]],
                        replica_groups=rg)

                def phase2_chunk(b, qc, Ss, ob_sb):
                    for h in range(HPC):
                        scores_group(b, h, Ss[h], qc)
                    for h in range(HPC):
                        pv_chunk(b, h, qc, Ss[h], ob_sb)
                    for (c0, c1) in AG_PIECES[b]:
                        if c1 == qc + 1:
                            ag_fire(b, c0, c1, ob_sb)

                # ---- block 1: v transposes interleaved with phase2(b=0) ----
                with tc.tile_pool(name="psT", bufs=2, space="PSUM") as psT:
                    def tr_group(th, eb2, tg):
                        tr = psT.tile([P, 512], BF, name="tr", tag="tr")
                        for j in range(4):
                            tt_g = th * N_LT + tg * 4 + j
                            nc.tensor.transpose(
                                tr[:, j * P:(j + 1) * P],
                                vt_sb[:, eb2 * T + tt_g * P:
                                      eb2 * T + (tt_g + 1) * P],
                                ident[:, :])
                        for j in range(4):
                            tt_g = th * N_LT + tg * 4 + j
                            nc.vector.tensor_copy(
                                v_sb[:, tt_g * ES + eb2 * P:
                                     tt_g * ES + (eb2 + 1) * P],
                                tr[:, j * P:(j + 1) * P])

                    for tg in range(4):           # batch-0 v tiles first
                        tr_group(0, 0, tg)
                        tr_group(0, 1, tg)
                    Ss0 = [pexp.tile([P, EXP_COLS], BF, name=f"S{h}", tag="S")
                           for h in range(HPC)]
                    ob_tiles[0] = p2ob.tile([P, HPC * L], BF,
                                            name="ob_sb", tag="ob")
                    for qc in range(4):
                        phase2_chunk(0, qc, Ss0, ob_tiles[0])
                        tr_group(1, 0, qc)        # batch-1 v tiles, spread out
                        tr_group(1, 1, qc)

                # ---- block 2: phase2(b=1) with Wo pieces interleaved ----
                with tc.tile_pool(name="psW", bufs=2, space="PSUM") as psW:
                    def phase3(b, c0, c1):
                        w = (c1 - c0) * 512
                        nch = w // 512
                        ots = []
                        for tch in range(nch):
                            ot = p3x.tile([P, N_DT * 512], BF,
                                          name="ot", tag="ot")
                            for dt in range(N_DT):
                                nc.sync.dma_start(
                                    out=ot[:, dt * 512:(dt + 1) * 512],
                                    in_=ag_o[(b, c0)][dt * P:(dt + 1) * P,
                                                      tch * 512:(tch + 1) * 512])
                            ots.append(ot)
                        for et in range(HPC):
                            fps = [psW.tile([P, 512], FP, name="f_ps", tag="f")
                                   for _ in range(nch)]
                            for dt in range(N_DT):
                                lhsT = woT_sb[:, dt * ES + et * P:
                                              dt * ES + (et + 1) * P]
                                for i in range(nch):
                                    nc.tensor.matmul(
                                        fps[i][:, :], lhsT,
                                        ots[i][:, dt * 512:(dt + 1) * 512],
                                        start=(dt == 0), stop=(dt == N_DT - 1))
                            for i in range(nch):
                                t0 = b * L + (c0 + i) * 512
                                f_sb = p3o.tile([P, 512], FP, name="f_sb")
                                nc.vector.tensor_copy(f_sb[:, :], fps[i][:, :])
                                nc.sync.dma_start(
                                    out=out[et * P:(et + 1) * P, t0:t0 + 512],
                                    in_=f_sb[:, :])

                    ph3_after = {0: [(0, 0, 2)], 1: [(0, 2, 4)],
                                 2: [(1, 0, 2)], 3: [(1, 2, 3), (1, 3, 4)]}
                    Ss1 = [pexp.tile([P, EXP_COLS], BF, name=f"S{h}", tag="S")
                           for h in range(HPC)]
                    ob_tiles[1] = p2ob.tile([P, HPC * L], BF,
                                            name="ob_sb", tag="ob")
                    for qc in range(4):
                        phase2_chunk(1, qc, Ss1, ob_tiles[1])
                        for args in ph3_after.get(qc, []):
                            phase3(*args)

                if debug:
                    for nm, src, dd in (("q", qt_sb, dbg_qt),
                                        ("k", kt_sb, dbg_kt),
                                        ("v", v_sb, dbg_v)):
                        for i in range(HPC * T // 512):
                            sdb = p3o.tile([P, 512], FP, name="f_sb")
                            nc.vector.tensor_copy(
                                sdb[:, :], src[:, i * 512:(i + 1) * 512])
                            nc.sync.dma_start(out=dd[:, i * 512:(i + 1) * 512],
                                              in_=sdb[:, :])
                    if True:
                        for b in range(B):
                            for i in range(HPC * L // 512):
                                s = p3o.tile([P, 512], FP, name="f_sb")
                                nc.vector.tensor_copy(
                                    s[:, :],
                                    ob_tiles[b][:, i * 512:(i + 1) * 512])
                                nc.sync.dma_start(
                                    out=dbg_ob[:, b * HPC * L + i * 512:
                                               b * HPC * L + (i + 1) * 512],
                                    in_=s[:, :])

    if fix_waits:
        split_multi_waits(nc)
    return nc


def make_in_maps(x, cos, sin, Wqkv, Wo):
    bf = ml_dtypes.bfloat16
    xT_full = np.ascontiguousarray(
        np.asarray(x).reshape(T, D).T).astype(bf)
    # q/k row permutation: head-major, evens then odds
    perm = []
    for h in range(HPC):
        perm.extend(h * HD + 2 * np.arange(64))
        perm.extend(h * HD + 2 * np.arange(64) + 1)
    perm = np.asarray(perm)
    in_maps = []
    cosA, sinA = np.asarray(cos), np.asarray(sin)
    for c in range(N_CORES):
        cols = slice(c * ES, (c + 1) * ES)
        wq = Wqkv[c * ES:(c + 1) * ES, :][perm]
        wk = Wqkv[D + c * ES: D + (c + 1) * ES, :][perm]
        wv = Wqkv[2 * D + c * ES: 2 * D + (c + 1) * ES, :]
        w_c = np.concatenate([wq, wk, wv], axis=0)
        m = {
            "xT": xT_full,
            "wqkvT": np.ascontiguousarray(w_c.T.astype(bf)),
            "woT": np.ascontiguousarray(Wo[cols, :].T.astype(bf)),
        }
        for h in range(HPC):
            base = c * ES + h * HD
            ce = cosA[:, base + 2 * np.arange(64)].T      # [64, L]
            co = cosA[:, base + 2 * np.arange(64) + 1].T
            se = sinA[:, base + 2 * np.arange(64)].T
            so = sinA[:, base + 2 * np.arange(64) + 1].T
            m[f"cc{h}"] = np.ascontiguousarray(
                np.concatenate([ce, co], axis=0)).astype(bf)
            m[f"ss{h}"] = np.ascontiguousarray(
                np.concatenate([se, so], axis=0)).astype(bf)
        in_maps.append(m)
    return in_maps


_cache = {}


def kernel(x, cos, sin, Wqkv, Wo):
    from concourse.bass_utils import run_bass_kernel_spmd
    x = np.asarray(x, dtype=np.float32)
    cos = np.asarray(cos, dtype=np.float32)
    sin = np.asarray(sin, dtype=np.float32)
    Wqkv = np.asarray(Wqkv, dtype=np.float32)
    Wo = np.asarray(Wo, dtype=np.float32)
    if "nc" not in _cache:
        _cache["nc"] = build()
    nc = _cache["nc"]
    in_maps = make_in_maps(x, cos, sin, Wqkv, Wo)
    res = run_bass_kernel_spmd(nc, in_maps, core_ids=list(range(N_CORES)))
    pieces = [res.results[c]["out"].T for c in range(N_CORES)]
    return np.concatenate(pieces, axis=1).reshape(B, L, D)


# revision 18
# speedup vs baseline: 1.1661x; 1.0621x over previous
"""Distributed causal attention block (QKV + RoPE + SDPA + Wo) on 8 TRN2 cores.

Sharding: tensor-parallel over heads (2 heads/core). Each core:
  phase 1: weight-stationary transposed QKV: q^T/k^T/v^T = Wqkv_c @ x^T
           streamed from host-pretransposed x^T (bf16); RoPE applied in the
           [e, t] layout with per-head even/odd partition split; v^T is
           PE-transposed back to [t, e] for the PV matmuls
  phase 2: causal attention per (batch, head) with TRANSPOSED scores
           s^T[k, q] (no P transposes); softmax sums via all-ones matmul
           (broadcast across partitions); 1/sum folded into the o^T drain
  phase 3: AllGather attention outputs (progressive pieces) -> Wo e-slice,
           emitted interleaved with phase 2 to avoid a serial tail
Host concatenates the 8 e-slices.

The q/k rows of Wqkv (and cos/sin tables) are permuted head-major
even/odd on the host; attention scores are invariant to a shared
permutation of the head dim of Q and K.
"""
import numpy as np
import ml_dtypes
import bass_rust
import concourse.bass as bass
import concourse.mybir as mybir
from concourse.tile import TileContext, add_dep_helper
from concourse.masks import make_identity

B, L, D, H = 2, 2048, 2048, 16
HD = 128
N_CORES = 8
HPC = H // N_CORES          # heads per core = 2
ES = HPC * HD               # 256 = e-slice width per core
T = B * L                   # 4096 tokens total
P = 128
SCALE = 1.0 / float(np.sqrt(HD))
NEG = -30000.0              # causal mask fill; exp(SCALE*(s+NEG)) underflows to 0
FP = mybir.dt.float32
BF = mybir.dt.bfloat16

N_TT = T // P               # 32 global t-tiles
N_LT = L // P               # 16 t-tiles per batch
N_DT = D // P               # 16 d-tiles
NQC = 4                     # 512-token q-chunks per batch

# attention-out AllGather pieces per batch, in units of 512-t q-chunks
AG_PIECES = {0: [(0, 2), (2, 4)], 1: [(0, 2), (2, 3), (3, 4)]}

# phase-2 block geometry: for (kt, qc) with qc >= kt//4:
#   off  = max(0, kt*128 - qc*512)   (column offset within the qc chunk)
#   w    = 512 - off
# blocks laid out kt-major in the expST tile
BLOCKS = []          # (kt, qc, off, w, boff)
_boff = 0
for _kt in range(16):
    for _qc in range(_kt // 4, 4):
        _off = max(0, _kt * 128 - _qc * 512)
        _w = 512 - _off
        BLOCKS.append((_kt, _qc, _off, _w, _boff))
        _boff += _w
EXP_COLS = _boff     # 17408
BLK = {(kt, qc): (off, w, boff) for (kt, qc, off, w, boff) in BLOCKS}


def split_multi_waits(nc):
    """This walrus build allows 1 sync wait per instruction (2 for
    EventSemaphore). Tile attaches more on some instructions (tail drain,
    collective-adjacent DMAs); hoist the extras onto same-engine NoOps."""
    for f in nc.m.functions:
        for bb in f.blocks:
            new_insts = []
            changed = False
            for ins in bb.instructions:
                si = ins.sync_info
                cap = 2 if type(ins).__name__ == "InstEventSemaphore" else 1
                if si is not None and len(si.on_wait) > cap:
                    waits = list(si.on_wait)
                    for k, w in enumerate(waits[cap:]):
                        new_insts.append(mybir.InstNoOp(
                            name=f"{ins.name}-wsplit{k}", ins=[], outs=[],
                            engine=ins.engine,
                            sync_info=bass_rust.SyncInfo(on_wait=[w], on_update=[]),
                        ))
                    ins.sync_info = bass_rust.SyncInfo(
                        on_wait=waits[:cap], on_update=list(si.on_update))
                    changed = True
                new_insts.append(ins)
            if changed:
                bb.instructions.clear()
                for i2 in new_insts:
                    bb.add_instruction(i2)


def make_causal_mask_T(nc, ap, mask_val):
    """mask[k, q] = 0 if k <= q else mask_val (transposed causal)."""
    sq = ap.shape[0]
    nc.gpsimd.memset(ap, 0.0)
    nc.gpsimd.affine_select(
        out=ap, in_=ap,
        compare_op=mybir.AluOpType.is_ge,
        fill=mask_val, base=0,
        # keep where (-x + y) >= 0, i.e. k <= q
        pattern=[[1, sq]],
        channel_multiplier=-1,
    )


def build(debug=False, fix_waits=True):
    nc = bass.Bass()
    xT = nc.declare_dram_parameter("xT", [D, T], BF, isOutput=False)
    wqkvT = nc.declare_dram_parameter("wqkvT", [D, 3 * ES], BF, isOutput=False)
    # per-head stacked trig tables: rows 0:64 = even-col table, 64:128 = odd
    cc_p = [nc.declare_dram_parameter(f"cc{h}", [P, L], BF, isOutput=False)
            for h in range(HPC)]
    ss_p = [nc.declare_dram_parameter(f"ss{h}", [P, L], BF, isOutput=False)
            for h in range(HPC)]
    woT = nc.declare_dram_parameter("woT", [D, ES], BF, isOutput=False)
    out = nc.declare_dram_parameter("out", [ES, T], FP, isOutput=True)
    if debug:
        dbg_qt = nc.declare_dram_parameter("dbg_qt", [P, HPC * T], FP, isOutput=True)
        dbg_kt = nc.declare_dram_parameter("dbg_kt", [P, HPC * T], FP, isOutput=True)
        dbg_v = nc.declare_dram_parameter("dbg_v", [P, N_TT * ES], FP, isOutput=True)
        dbg_ob = nc.declare_dram_parameter("dbg_ob", [P, B * HPC * L], FP,
                                           isOutput=True)

    o_bounce, ag_o = {}, {}
    for b, pieces in AG_PIECES.items():
        for (c0, c1) in pieces:
            w = (c1 - c0) * 512
            o_bounce[(b, c0)] = nc.dram_tensor(f"o_bounce{b}_{c0}", [ES, w], BF)
            ag_o[(b, c0)] = nc.dram_tensor(f"ag_o{b}_{c0}", [N_CORES * ES, w], BF,
                                           addr_space="Shared")
    rg = [list(range(N_CORES))]

    with TileContext(nc, pool_alloc_mode="queue") as tc:
        with (
            tc.tile_pool(name="const", bufs=1) as const_pool,
            tc.tile_pool(name="resident", bufs=1) as res_pool,
            tc.tile_pool(name="wo", bufs=1) as wo_pool,
            tc.tile_pool(name="vt", bufs=1) as vt_pool,
        ):
            ident = const_pool.tile([P, P], BF, name="ident")
            make_identity(nc, ident[:, :])
            ones = const_pool.tile([P, P], BF, name="ones")
            nc.gpsimd.memset(ones[:, :], 1.0)
            tri = const_pool.tile([P, P], BF, name="tri")
            nc.gpsimd.memset(tri[:, :], 1.0)
            nc.gpsimd.affine_select(
                out=tri[:, :], in_=tri[:, :],
                compare_op=mybir.AluOpType.is_ge, fill=0.0, base=0,
                pattern=[[1, P]], channel_multiplier=-1)

            # resident through phases 1-2
            qt_sb = res_pool.tile([P, HPC * T], BF, name="qt_sb")   # [hd', h*T+t]
            kt_sb = res_pool.tile([P, HPC * T], BF, name="kt_sb")
            v_sb = res_pool.tile([P, N_TT * ES], BF, name="v_sb")   # [t%128, tt*ES+e]
            woT_sb = wo_pool.tile([P, N_DT * ES], BF, name="woT_sb")

            # ---------------- phase 1: transposed QKV + RoPE ----------------
            # eb order: q-h0, q-h1, k-h0, k-h1, v-0, v-1
            with (
                tc.tile_pool(name="wq", bufs=1) as wq_pool,
                tc.tile_pool(name="xt", bufs=1) as xt_pool,
                tc.tile_pool(name="rsc", bufs=1) as rsc_pool,
                tc.tile_pool(name="psG", bufs=2, space="PSUM") as psG,
            ):
                wt_sb = wq_pool.tile([P, N_DT * 3 * ES], BF, name="wt_sb")
                cc_sb = [wq_pool.tile([P, L], BF, name=f"cc{h}_sb")
                         for h in range(HPC)]
                ss_sb = [wq_pool.tile([P, L], BF, name=f"ss{h}_sb")
                         for h in range(HPC)]
                xt_sb = xt_pool.tile([P, N_DT * 2048], BF, name="xt_sb")
                vt_sb = vt_pool.tile([P, HPC * T], BF,
                                     name="vt_sb")  # [e, eb*T + t]

                # DMA priority: x^T th0 tiles + weights interleaved, then trig
                for dt in range(N_DT):
                    nc.sync.dma_start(
                        out=xt_sb[:, dt * 2048:(dt + 1) * 2048],
                        in_=xT[dt * P:(dt + 1) * P, 0:2048])
                    nc.sync.dma_start(
                        out=wt_sb[:, dt * 3 * ES:(dt + 1) * 3 * ES],
                        in_=wqkvT[dt * P:(dt + 1) * P, :])
                for h in range(HPC):
                    nc.sync.dma_start(out=cc_sb[h][:, :], in_=cc_p[h][:, :])
                    nc.sync.dma_start(out=ss_sb[h][:, :], in_=ss_p[h][:, :])

                def rope_drain(gp, dst, h, th):
                    cc, ss = cc_sb[h], ss_sb[h]
                    dcol = slice(h * T + th * 2048, h * T + (th + 1) * 2048)
                    e_ps, o_ps = gp[0:64, :], gp[64:128, :]
                    t1 = rsc_pool.tile([64, 2048], FP, name="t1", tag="t1")
                    t2 = rsc_pool.tile([64, 2048], FP, name="t2", tag="t2")
                    nc.vector.tensor_tensor(t1[:, :], e_ps, cc[0:64, :],
                                            op=mybir.AluOpType.mult)
                    nc.vector.tensor_tensor(t2[:, :], o_ps, ss[0:64, :],
                                            op=mybir.AluOpType.mult)
                    nc.vector.tensor_tensor(dst[0:64, dcol], t1[:, :], t2[:, :],
                                            op=mybir.AluOpType.subtract)
                    t3 = rsc_pool.tile([64, 2048], FP, name="t3", tag="t1")
                    t4 = rsc_pool.tile([64, 2048], FP, name="t4", tag="t2")
                    nc.vector.tensor_tensor(t3[:, :], o_ps, cc[64:128, :],
                                            op=mybir.AluOpType.mult)
                    nc.vector.tensor_tensor(t4[:, :], e_ps, ss[64:128, :],
                                            op=mybir.AluOpType.mult)
                    nc.vector.tensor_tensor(dst[64:128, dcol], t3[:, :], t4[:, :],
                                            op=mybir.AluOpType.add)

                EB_ORDER = [4, 5, 0, 1, 2, 3]   # v first; tables can lag
                for th in range(2):
                    for i, ebi in enumerate(EB_ORDER):
                        gp = psG.tile([P, 2048], FP, name="gp", tag="gp")
                        for dt in range(N_DT):
                            lhsT = wt_sb[:, dt * 3 * ES + ebi * P:
                                         dt * 3 * ES + (ebi + 1) * P]
                            for c in range(4):
                                nc.tensor.matmul(
                                    gp[:, c * 512:(c + 1) * 512], lhsT,
                                    xt_sb[:, dt * 2048 + c * 512:
                                          dt * 2048 + (c + 1) * 512],
                                    start=(dt == 0), stop=(dt == N_DT - 1))
                            if th == 0 and i == 5:
                                nc.sync.dma_start(
                                    out=xt_sb[:, dt * 2048:(dt + 1) * 2048],
                                    in_=xT[dt * P:(dt + 1) * P, 2048:4096])
                        if ebi < 2:
                            rope_drain(gp, qt_sb, ebi, th)
                        elif ebi < 4:
                            rope_drain(gp, kt_sb, ebi - 2, th)
                        else:
                            eb2 = ebi - 4
                            nc.scalar.copy(
                                vt_sb[:, eb2 * T + th * 2048:
                                      eb2 * T + (th + 1) * 2048],
                                gp[:, :])

            # ---------------- phases 2+3 (interleaved) ----------------
            for dt in range(N_DT):
                nc.sync.dma_start(out=woT_sb[:, dt * ES:(dt + 1) * ES],
                                  in_=woT[dt * P:(dt + 1) * P, :])

            with (
                tc.tile_pool(name="pexp", bufs=2) as pexp,
                tc.tile_pool(name="prec", bufs=2) as prec,
                tc.tile_pool(name="p2ob", bufs=2) as p2ob,
                tc.tile_pool(name="p3x", bufs=2) as p3x,
                tc.tile_pool(name="p3o", bufs=2) as p3o,
                tc.tile_pool(name="psS", bufs=3, space="PSUM") as psS,
                tc.tile_pool(name="psSum", bufs=1, space="PSUM") as psSum,
                tc.tile_pool(name="psO", bufs=2, space="PSUM") as psO,
            ):
                ob_tiles = {}

                def scores_group(b, h, S, ktg):
                    """score blocks for k-tiles ktg*4..ktg*4+3 (kt-major)."""
                    qoff = h * T + b * L
                    for kt in range(ktg * 4, ktg * 4 + 4):
                        lhsT = kt_sb[:, qoff + kt * P: qoff + (kt + 1) * P]
                        for qc in range(kt // 4, 4):
                            off, w, boff = BLK[(kt, qc)]
                            sp = psS.tile([P, 512], FP, name="sp", tag="sp")
                            nc.tensor.matmul(
                                sp[:, :w], lhsT,
                                qt_sb[:, qoff + qc * 512 + off:
                                      qoff + (qc + 1) * 512],
                                start=True, stop=True)
                            nc.scalar.activation(
                                S[:, boff:boff + w], sp[:, :w],
                                mybir.ActivationFunctionType.Exp, scale=SCALE)
                            if qc == kt // 4:  # zero masked (k>q) triangle
                                nc.vector.tensor_tensor(
                                    S[:, boff:boff + P], S[:, boff:boff + P],
                                    tri[:, :], op=mybir.AluOpType.mult)

                def pv_chunk(b, h, qc, S, ob_sb):
                    """softmax-normalize + PV for one 512-q chunk."""
                    nkt = 4 * qc + 4
                    # k-block partial sums accumulated on DVE (fp32), then a
                    # single all-ones matmul for the cross-partition reduce
                    fulls = [kt for kt in range(nkt) if BLK[(kt, qc)][0] == 0]
                    parts = [kt for kt in range(nkt) if BLK[(kt, qc)][0] > 0]
                    acc = prec.tile([P, 512], FP, name="acc", tag="acc")
                    if len(fulls) == 1:
                        bo = BLK[(fulls[0], qc)][2]
                        nc.vector.tensor_copy(acc[:, :], S[:, bo:bo + 512])
                    else:
                        bo0, bo1 = (BLK[(fulls[0], qc)][2],
                                    BLK[(fulls[1], qc)][2])
                        nc.vector.tensor_tensor(
                            acc[:, :], S[:, bo0:bo0 + 512],
                            S[:, bo1:bo1 + 512], op=mybir.AluOpType.add)
                        for kt in fulls[2:]:
                            bo = BLK[(kt, qc)][2]
                            nc.vector.tensor_tensor(
                                acc[:, :], acc[:, :], S[:, bo:bo + 512],
                                op=mybir.AluOpType.add)
                    for kt in parts:
                        off, w, bo = BLK[(kt, qc)]
                        nc.vector.tensor_tensor(
                            acc[:, off:], acc[:, off:], S[:, bo:bo + w],
                            op=mybir.AluOpType.add)
                    accb = prec.tile([P, 512], BF, name="accb", tag="accb")
                    nc.vector.tensor_copy(accb[:, :], acc[:, :])
                    sm = psSum.tile([P, 512], FP, name="sm", tag="sm")
                    nc.tensor.matmul(sm[:, :], ones[:, :], accb[:, :],
                                     start=True, stop=True)
                    lsm = prec.tile([P, 512], FP, name="lsm", tag="lsm")
                    nc.scalar.activation(lsm[:, :], sm[:, :],
                                         mybir.ActivationFunctionType.Ln)
                    rec = prec.tile([P, 512], FP, name="rec", tag="rec")
                    nc.scalar.activation(rec[:, :], lsm[:, :],
                                         mybir.ActivationFunctionType.Exp,
                                         scale=-1.0)
                    o_ps = psO.tile([P, 512], FP, name="o_ps", tag="o")
                    for kt in range(nkt):
                        off, w, boff = BLK[(kt, qc)]
                        nc.tensor.matmul(
                            o_ps[:, off:],
                            v_sb[:, (b * N_LT + kt) * ES + h * HD:
                                 (b * N_LT + kt) * ES + (h + 1) * HD],
                            S[:, boff:boff + w],
                            start=(kt == 0), stop=(kt == nkt - 1))
                    nc.vector.tensor_tensor(
                        ob_sb[:, h * L + qc * 512:h * L + (qc + 1) * 512],
                        o_ps[:, :], rec[:, :], op=mybir.AluOpType.mult)

                def ag_fire(b, c0, c1, ob_sb):
                    for h in range(HPC):
                        nc.sync.dma_start(
                            out=o_bounce[(b, c0)][h * HD:(h + 1) * HD, :],
                            in_=ob_sb[:, h * L + c0 * 512:h * L + c1 * 512])
                    nc.gpsimd.collective_compute(
                        "AllGather", mybir.AluOpType.bypass,
                        ins=[o_bounce[(b, c0)][:]],
                        outs=[ag_o[(b, c0)][:]],
                        replica_groups=rg)

                def phase2_chunk(b, qc, Ss, ob_sb):
                    for h in range(HPC):
                        scores_group(b, h, Ss[h], qc)
                    for h in range(HPC):
                        pv_chunk(b, h, qc, Ss[h], ob_sb)
                    for (c0, c1) in AG_PIECES[b]:
                        if c1 == qc + 1:
                            ag_fire(b, c0, c1, ob_sb)

                # ---- block 1: v transposes interleaved with phase2(b=0) ----
                with tc.tile_pool(name="psT", bufs=2, space="PSUM") as psT:
                    def tr_group(th, eb2, tg):
                        tr = psT.tile([P, 512], BF, name="tr", tag="tr")
                        for j in range(4):
                            tt_g = th * N_LT + tg * 4 + j
                            nc.tensor.transpose(
                                tr[:, j * P:(j + 1) * P],
                                vt_sb[:, eb2 * T + tt_g * P:
                                      eb2 * T + (tt_g + 1) * P],
                                ident[:, :])
                        for j in range(4):
                            tt_g = th * N_LT + tg * 4 + j
                            nc.vector.tensor_copy(
                                v_sb[:, tt_g * ES + eb2 * P:
                                     tt_g * ES + (eb2 + 1) * P],
                                tr[:, j * P:(j + 1) * P])

                    for tg in range(4):           # batch-0 v tiles first
                        tr_group(0, 0, tg)
                        tr_group(0, 1, tg)
                    Ss0 = [pexp.tile([P, EXP_COLS], BF, name=f"S{h}", tag="S")
                           for h in range(HPC)]
                    ob_tiles[0] = p2ob.tile([P, HPC * L], BF,
                                            name="ob_sb", tag="ob")
                    for qc in range(4):
                        phase2_chunk(0, qc, Ss0, ob_tiles[0])
                        tr_group(1, 0, qc)        # batch-1 v tiles, spread out
                        tr_group(1, 1, qc)

                # ---- block 2: phase2(b=1) with Wo pieces interleaved ----
                with tc.tile_pool(name="psW", bufs=2, space="PSUM") as psW:
                    def phase3(b, c0, c1):
                        w = (c1 - c0) * 512
                        nch = w // 512
                        ots = []
                        for tch in range(nch):
                            ot = p3x.tile([P, N_DT * 512], BF,
                                          name="ot", tag="ot")
                            for dt in range(N_DT):
                                nc.sync.dma_start(
                                    out=ot[:, dt * 512:(dt + 1) * 512],
                                    in_=ag_o[(b, c0)][dt * P:(dt + 1) * P,
                                                      tch * 512:(tch + 1) * 512])
                            ots.append(ot)
                        for et in range(HPC):
                            fps = [psW.tile([P, 512], FP, name="f_ps", tag="f")
                                   for _ in range(nch)]
                            for dt in range(N_DT):
                                lhsT = woT_sb[:, dt * ES + et * P:
                                              dt * ES + (et + 1) * P]
                                for i in range(nch):
                                    nc.tensor.matmul(
                                        fps[i][:, :], lhsT,
                                        ots[i][:, dt * 512:(dt + 1) * 512],
                                        start=(dt == 0), stop=(dt == N_DT - 1))
                            for i in range(nch):
                                t0 = b * L + (c0 + i) * 512
                                f_sb = p3o.tile([P, 512], FP, name="f_sb")
                                nc.vector.tensor_copy(f_sb[:, :], fps[i][:, :])
                                nc.sync.dma_start(
                                    out=out[et * P:(et + 1) * P, t0:t0 + 512],
                                    in_=f_sb[:, :])

                    ph3_after = {0: [(0, 0, 2)], 1: [(0, 2, 4)],
                                 2: [(1, 0, 2)], 3: [(1, 2, 3), (1, 3, 4)]}
                    Ss1 = [pexp.tile([P, EXP_COLS], BF, name=f"S{h}", tag="S")
                           for h in range(HPC)]
                    ob_tiles[1] = p2ob.tile([P, HPC * L], BF,
                                            name="ob_sb", tag="ob")
                    for qc in range(4):
                        phase2_chunk(1, qc, Ss1, ob_tiles[1])
                        for args in ph3_after.get(qc, []):
                            phase3(*args)

                if debug:
                    for nm, src, dd in (("q", qt_sb, dbg_qt),
                                        ("k", kt_sb, dbg_kt),
                                        ("v", v_sb, dbg_v)):
                        for i in range(HPC * T // 512):
                            sdb = p3o.tile([P, 512], FP, name="f_sb")
                            nc.vector.tensor_copy(
                                sdb[:, :], src[:, i * 512:(i + 1) * 512])
                            nc.sync.dma_start(out=dd[:, i * 512:(i + 1) * 512],
                                              in_=sdb[:, :])
                    if True:
                        for b in range(B):
                            for i in range(HPC * L // 512):
                                s = p3o.tile([P, 512], FP, name="f_sb")
                                nc.vector.tensor_copy(
                                    s[:, :],
                                    ob_tiles[b][:, i * 512:(i + 1) * 512])
                                nc.sync.dma_start(
                                    out=dbg_ob[:, b * HPC * L + i * 512:
                                               b * HPC * L + (i + 1) * 512],
                                    in_=s[:, :])

    if fix_waits:
        split_multi_waits(nc)
    return nc


def make_in_maps(x, cos, sin, Wqkv, Wo):
    bf = ml_dtypes.bfloat16
    xT_full = np.ascontiguousarray(
        np.asarray(x).reshape(T, D).T).astype(bf)
    # q/k row permutation: head-major, evens then odds
    perm = []
    for h in range(HPC):
        perm.extend(h * HD + 2 * np.arange(64))
        perm.extend(h * HD + 2 * np.arange(64) + 1)
    perm = np.asarray(perm)
    in_maps = []
    cosA, sinA = np.asarray(cos), np.asarray(sin)
    for c in range(N_CORES):
        cols = slice(c * ES, (c + 1) * ES)
        wq = Wqkv[c * ES:(c + 1) * ES, :][perm]
        wk = Wqkv[D + c * ES: D + (c + 1) * ES, :][perm]
        wv = Wqkv[2 * D + c * ES: 2 * D + (c + 1) * ES, :]
        w_c = np.concatenate([wq, wk, wv], axis=0)
        m = {
            "xT": xT_full,
            "wqkvT": np.ascontiguousarray(w_c.T.astype(bf)),
            "woT": np.ascontiguousarray(Wo[cols, :].T.astype(bf)),
        }
        for h in range(HPC):
            base = c * ES + h * HD
            ce = cosA[:, base + 2 * np.arange(64)].T      # [64, L]
            co = cosA[:, base + 2 * np.arange(64) + 1].T
            se = sinA[:, base + 2 * np.arange(64)].T
            so = sinA[:, base + 2 * np.arange(64) + 1].T
            m[f"cc{h}"] = np.ascontiguousarray(
                np.concatenate([ce, co], axis=0)).astype(bf)
            m[f"ss{h}"] = np.ascontiguousarray(
                np.concatenate([se, so], axis=0)).astype(bf)
        in_maps.append(m)
    return in_maps


_cache = {}


def kernel(x, cos, sin, Wqkv, Wo):
    from concourse.bass_utils import run_bass_kernel_spmd
    x = np.asarray(x, dtype=np.float32)
    cos = np.asarray(cos, dtype=np.float32)
    sin = np.asarray(sin, dtype=np.float32)
    Wqkv = np.asarray(Wqkv, dtype=np.float32)
    Wo = np.asarray(Wo, dtype=np.float32)
    if "nc" not in _cache:
        _cache["nc"] = build()
    nc = _cache["nc"]
    in_maps = make_in_maps(x, cos, sin, Wqkv, Wo)
    res = run_bass_kernel_spmd(nc, in_maps, core_ids=list(range(N_CORES)))
    pieces = [res.results[c]["out"].T for c in range(N_CORES)]
    return np.concatenate(pieces, axis=1).reshape(B, L, D)
